# revision 33
# baseline (speedup 1.0000x reference)
"""DANet3D dual-attention kernel for Trainium2 (8 NeuronCores, Bass/Tile).

Sharding: x -> proj p [2, 64, 8000]; 8 cores = 2 batches x 4 query-blocks
of 2000 positions.  Each core receives the full batch projection (keys /
values / channel attention) plus its own query block and computes its
[64, 2000] slice of the output.

Position attention (per batch), with M = Wq^T Wk, w = Wk^T bq:
  softmax_m( p_n^T M p_m + w.p_m )  ->  flash loop in E^T layout
  F = exp(kp_m . p_n + w.p_m),  kp = M p
  U[65, q] += vt[m, 0:65]^T F[m, q],  vt = [gamma_p*vT | 1 | w.p]

v2 pipeline: the exp of the 8064x2000 score matrix is the bottleneck
(ACT ~1.1ns/col at FD=1012 vs ~1.6 at FD=500; DVE ~1.2), so the loop is
restructured for 1012-column exp instructions:
  * queries are processed in two phases of 1000 (chunks 0,1 then 2,3),
    so U needs only 2 PSUM banks and F gets 6 (three 2-bank pairs);
  * per sub-iter s = (phase, pair i): 4 F matmuls write key-tile i's two
    500-col chunks into one 2-bank pair and tile 32+i's into another
    (h0/h64 row groups run the a/b matmuls concurrently);
  * one ACT exp (bias slot) and one DVE Schraudolph (vec scalar2) each
    cover a full 1012-col pair span; roles alternate per sub-iter;
  * the channel-attention softmax/oc runs right after the Gram prologue
    in bf16 (PE transpose + bf16 oc matmuls) instead of a serialized
    fp32 epilogue; phase-0 U banks drain mid-flash.
"""

from contextlib import ExitStack

import ml_dtypes
import numpy as np

import concourse.bass as bass
import concourse.mybir as mybir
import concourse.tile as tile
from concourse import bacc
from concourse.bass import ds, ts
from concourse.bass_utils import run_bass_kernel_spmd
from concourse.masks import make_identity

F32 = mybir.dt.float32
BF16 = mybir.dt.bfloat16
I16 = mybir.dt.int16
AF = mybir.ActivationFunctionType
ALU = mybir.AluOpType
AX = mybir.AxisListType

B, C, D, H, W = 2, 64, 20, 20, 20
N = D * H * W            # 8000
MT = 128                 # key (m) tile size
NRT = 63                 # real m tiles (63*128 = 8064 >= 8000)
NPAD = 8192              # padded key range in pab
HALF = NPAD // 2         # 4096 (m-tile pair split)
NPAIR = 32               # pair iterations (A=i, B=32+i)
NQ = 2000                # queries per core
CH = 500                 # query chunk width (4 chunks)
KCH = 512                # kp projection chunk
LAVT = 4                 # vt pair lookahead
NCORES = 8
SCH_C = 184.6650390625   # 128/ln(2): bf16 Schraudolph scale
SCH_B = 16256.0          # 127*128
NSUB = 64                # 2 phases x 32 pairs


def build_danet(ctx, tc, io):
    nc = tc.nc
    xqb2, ptd = io["xqb2"], io["ptd"]
    kp2d, vtd, wpcbd = io["kp2d"], io["vtd"], io["wpcbd"]
    gc, eye2, out_d = io["gc"], io["eye2"], io["out"]

    persist = ctx.enter_context(tc.tile_pool(name="persist", bufs=1))
    fs_pool = ctx.enter_context(tc.tile_pool(name="fs", bufs=6))
    up = ctx.enter_context(tc.tile_pool(name="ps_u", bufs=1, space="PSUM"))
    fp = ctx.enter_context(tc.tile_pool(name="ps_f", bufs=1, space="PSUM"))

    paqb2 = persist.tile([128, NQ], BF16)     # query block bf16, duplicated
    kp2 = persist.tile([128, HALF], BF16)     # M@p packed halves
    vt = persist.tile([128, NRT, 66], BF16)   # [gamma_p*vT | 1 | w.p]
    pt = persist.tile([128, NRT, 64], BF16)   # projT tiles (DMA, channel)
    wpcb = persist.tile([128, NRT], F32)      # w.p*C + B (Schraudolph bias)
    gc_s = persist.tile([64, 1], F32)
    eye2_s = persist.tile([64, 64], F32)
    ones_s = persist.tile([1, 64], BF16)
    ec_acc = persist.tile([64, 64], F32)
    ee = persist.tile([64, 64], F32)
    eesc = persist.tile([64, 64], F32)
    id64 = persist.tile([64, 64], F32)
    ac2 = persist.tile([64, 64], BF16)
    mx = persist.tile([64, 1], F32)
    sc = persist.tile([64, 1], F32)
    rc = persist.tile([64, 1], F32)
    rcg = persist.tile([64, 1], F32)
    oc_sb = persist.tile([64, NQ], F32)       # gamma_c*out_c + 2x
    d4 = persist.tile([1, NQ], BF16)          # softmax denominators
    rcp = persist.tile([64, 2 * 512], F32)    # 1/denom bcast (ping-pong)
    out_sb = persist.tile([64, NQ], F32)
    u_sb = [persist.tile([65, 512], F32, name=f"u_sb{k}") for k in range(2)]

    # ---- input DMAs: minimal critical prefix first on each queue so
    # F(0)/U(0)/exp(0) can start within ~2us; bulk follows. ----
    nvt = NRT * 66
    # critical set on the sync (hardware-DGE) queue in need-order: the
    # software-DGE scalar/gpsimd queues have a long descriptor-gen
    # startup that used to gate the flash start at ~12us
    nc.sync.dma_start(out=kp2[:, 0:128], in_=kp2d[:, 0:128])
    nc.sync.dma_start(out=paqb2[0:64, 0:1000], in_=xqb2[0:64, 0:1000])
    nc.sync.dma_start(out=paqb2[64:128, 0:1000], in_=xqb2[64:128, 0:1000])
    nc.sync.dma_start(out=wpcb, in_=wpcbd)
    nc.sync.dma_start(out=vt[:, 0:4, :], in_=vtd[:, 0:4 * 66])
    nc.sync.dma_start(out=kp2[:, 128:1024], in_=kp2d[:, 128:1024])
    for i in range(1, 4):
        nc.sync.dma_start(out=kp2[:, ts(i, 1024)], in_=kp2d[:, ts(i, 1024)])
    nc.scalar.dma_start(out=paqb2[0:64, 1000:2000], in_=xqb2[0:64, 1000:2000])
    nc.scalar.dma_start(out=vt[:, 4:24, :], in_=vtd[:, 4 * 66:24 * 66])
    nc.scalar.dma_start(out=vt[:, 24:NRT, :], in_=vtd[:, 24 * 66:nvt])
    nc.gpsimd.dma_start(out=paqb2[64:128, 1000:2000],
                        in_=xqb2[64:128, 1000:2000])
    # pt quartered so the Gram burst can start as soon as tiles land
    for q in range(4):
        qt = 16 if q < 3 else NRT - 48
        nc.gpsimd.dma_start(out=pt[:, ds(16 * q, qt), :],
                            in_=ptd[:, ds(16 * q * 64, qt * 64)])
    nc.gpsimd.dma_start(out=gc_s, in_=gc)
    nc.gpsimd.dma_start(out=eye2_s, in_=eye2)
    make_identity(nc, id64)
    nc.vector.memset(ones_s, 1.0)

    # ---- HAM warm-up: ~4us of tiny matmuls during the input-DMA window
    # so the flash loop starts at K=8/8 (2.4 GHz) instead of 1.2 ----
    warm = fp.tile([128, 1024], F32, name="warm", tag="fp2")
    for r in range(50):
        nc.tensor.matmul(warm[0:64, 0:64], ones_s, ones_s,
                         start=True, stop=True)

    tag_n = [0]

    def tagf():
        tag_n[0] = (tag_n[0] + 1) % 3
        return f"fp{tag_n[0]}"

    gch = {}

    def emit_gram_channel(stage):
        """Channel attention in 4 stages to amortize PSUM tag steals."""
        if stage == 10:   # Gram burst (PE) + row softmax pieces
            g_ps = fp.tile([128, 1024], F32, name="gram", tag=tagf())
            for t in range(NRT):
                nc.tensor.matmul(g_ps[0:64, 0:64], pt[:, t, :], pt[:, t, :],
                                 start=(t == 0), stop=(t == NRT - 1))
            nc.vector.tensor_copy(out=ec_acc, in_=g_ps[0:64, 0:64])
            nc.vector.tensor_reduce(out=mx, in_=ec_acc, axis=AX.X,
                                    op=ALU.max, negate=True)
            nc.scalar.activation(out=ee, in_=ec_acc, func=AF.Exp, bias=mx)
            nc.vector.tensor_reduce(out=sc, in_=ee, axis=AX.X, op=ALU.add)
            nc.vector.reciprocal(out=rc, in_=sc)
            nc.vector.tensor_mul(out=rcg, in0=rc, in1=gc_s)
            nc.vector.tensor_scalar_mul(out=eesc, in0=ee, scalar1=rcg)
        elif stage == 12:  # transpose + ac2
            at_ps = fp.tile([128, 1024], F32, name="at_ps", tag=tagf())
            nc.tensor.transpose(at_ps[0:64, 0:64], eesc, id64)
            nc.vector.tensor_add(out=ac2, in0=at_ps[0:64, 0:64], in1=eye2_s)
        else:              # oc chunk pairs (bf16 matmuls, 2 per stage)
            c0 = 0 if stage == 14 else 2
            oc_ps = fp.tile([128, 1024], F32, name=f"oc{c0}", tag=tagf())
            for k in range(2):
                nc.tensor.matmul(oc_ps[0:64, ds(k * 512, CH)], ac2,
                                 paqb2[0:64, ts(c0 + k, CH)],
                                 start=True, stop=True)
            nc.scalar.copy(out=oc_sb[:, ts(c0, CH)],
                           in_=oc_ps[0:64, 0:CH])
            nc.vector.tensor_copy(out=oc_sb[:, ts(c0 + 1, CH)],
                                  in_=oc_ps[0:64, 512:512 + CH])

        # ---- main flash loop: 64 sub-iters = 2 phases x 32 pairs ----
    fps = [None] * NSUB   # (fa_tile, fb_tile)
    fsb = [None] * NSUB   # (fe_a, fe_b) exp outputs
    u_cur = [None, None]

    def emit_F(s):
        h, i = divmod(s, NPAIR)
        has_b = 32 + i <= NRT - 1
        fa = fp.tile([128, 1024], F32, name=f"fa{s}", tag=f"fp{(2 * s) % 3}")
        fb = fp.tile([128, 1024], F32, name=f"fb{s}",
                     tag=f"fp{(2 * s + 1) % 3}") if has_b else None
        for k in range(2):  # chunk c = 2h + k -> tile cols k*512
            qs = ds((2 * h + k) * CH, CH)
            nc.tensor.matmul(fa[:, ds(k * 512, CH)], kp2[0:64, ts(i, MT)],
                             paqb2[0:64, qs], start=True, stop=True,
                             tile_position=(0, 0))
            if has_b:
                nc.tensor.matmul(fb[:, ds(k * 512, CH)],
                                 kp2[64:128, ts(i, MT)], paqb2[64:128, qs],
                                 start=True, stop=True,
                                 tile_position=(64, 0))
        fps[s] = (fa, fb)

    def emit_exp(s):
        h, i = divmod(s, NPAIR)
        fa, fb = fps[s]
        a_on_act = True  # fixed roles: ACT frees fa tags, DVE fb tags
        outs = []
        for t, f_ps, on_act in ((2 * i, fa, a_on_act),
                                (2 * i + 1, fb, not a_on_act)):
            if f_ps is None:
                outs.append(None)
                continue
            if on_act:
                fe = fs_pool.tile([128, 1024], BF16, name="fsb", tag="fsb")
                nc.scalar.activation(out=fe[:, 0:1012], in_=f_ps[:, 0:1012],
                                     func=AF.Exp, bias=vt[:, t, 65:66])
                outs.append(fe)
            else:
                fe = fs_pool.tile([128, 1024], I16, name="fsb", tag="fsb")
                nc.vector.tensor_scalar(
                    out=fe[:, 0:1012], in0=f_ps[:, 0:1012],
                    scalar1=SCH_C, scalar2=wpcb[:, t:t + 1],
                    op0=ALU.mult, op1=ALU.add)
                outs.append(fe.bitcast(BF16))
        fsb[s] = outs
        fps[s] = None

    def emit_U(s):
        h, i = divmod(s, NPAIR)
        ea, eb = fsb[s]
        if i == 0:  # new phase: fresh U tiles on the shared 2 banks
            u_cur[0] = up.tile([65, 512], F32, name=f"u{h}0", tag="uu0")
            u_cur[1] = up.tile([65, 512], F32, name=f"u{h}1", tag="uu1")
        for k in range(2):  # same lhsT back-to-back: one LDWEIGHTS per tile
            nc.tensor.matmul(u_cur[k][:, 0:CH], vt[:, 2 * i, 0:65],
                             ea[:, ds(k * 512, CH)],
                             start=(i == 0), stop=(i == NPAIR - 1))
        if eb is not None:
            for k in range(2):
                nc.tensor.matmul(u_cur[k][:, 0:CH], vt[:, 2 * i + 1, 0:65],
                                 eb[:, ds(k * 512, CH)],
                                 start=False, stop=False)
        fsb[s] = None

    def emit_combine_mid(chunk, u_tile):
        """Phase-0 drain: copy U to SBUF fast, finish on GPSIMD."""
        csl = ds(chunk * CH, CH)
        nc.scalar.copy(out=d4[:, csl], in_=u_tile[64:65, 0:CH])
        usb = u_sb[chunk]
        if chunk % 2 == 0:
            nc.scalar.copy(out=usb[:, 0:CH], in_=u_tile[:, 0:CH])
        else:
            nc.vector.tensor_copy(out=usb[:, 0:CH], in_=u_tile[:, 0:CH])
        bc_ps = fp.tile([128, 1024], F32, name=f"bc{chunk}", tag=tagf())
        nc.tensor.matmul(bc_ps[0:64, 0:CH], ones_s, d4[:, csl],
                         start=True, stop=True)
        rsl = ds((chunk % 2) * 512, CH)
        nc.vector.reciprocal_approx_fast(out=rcp[:, rsl],
                                         in_=bc_ps[0:64, 0:CH])
        nc.gpsimd.tensor_tensor(out=out_sb[:, csl], in0=usb[0:64, 0:CH],
                                in1=rcp[:, rsl], op=ALU.mult)
        nc.gpsimd.tensor_tensor(out=out_sb[:, csl], in0=out_sb[:, csl],
                                in1=oc_sb[:, csl], op=ALU.add)
        nc.sync.dma_start(out=out_d[:, csl], in_=out_sb[:, csl])

    def emit_combine_end(chunk, u_tile):
        """Tail combine straight from PSUM on DVE."""
        csl = ds(chunk * CH, CH)
        nc.scalar.copy(out=d4[:, csl], in_=u_tile[64:65, 0:CH])
        bc_ps = fp.tile([128, 1024], F32, name=f"bc{chunk}", tag=tagf())
        nc.tensor.matmul(bc_ps[0:64, 0:CH], ones_s, d4[:, csl],
                         start=True, stop=True)
        rsl = ds((chunk % 2) * 512, CH)
        nc.vector.reciprocal_approx_fast(out=rcp[:, rsl],
                                         in_=bc_ps[0:64, 0:CH])
        nc.vector.tensor_mul(out=out_sb[:, csl], in0=u_tile[0:64, 0:CH],
                             in1=rcp[:, rsl])
        nc.vector.tensor_add(out=out_sb[:, csl], in0=out_sb[:, csl],
                             in1=oc_sb[:, csl])
        nc.sync.dma_start(out=out_d[:, csl], in_=out_sb[:, csl])

    u_done = [None] * 2   # phase-0 U tiles pending combine

    for step in range(NSUB + 2):
        jf, jx, ju = step, step - 1, step - 2
        if jf < NSUB:
            emit_F(jf)
        if 0 <= ju < NSUB:
            if ju == NPAIR:  # phase 1 begins: drain phase-0 U banks
                emit_combine_mid(0, u_done[0])
                emit_combine_mid(1, u_done[1])
            emit_U(ju)
            if ju % NPAIR == NPAIR - 1:
                u_done[0], u_done[1] = u_cur[0], u_cur[1]
        if 0 <= jx < NSUB:
            emit_exp(jx)
        if jf in (10, 12, 14, 16):
            emit_gram_channel(jf)

    emit_combine_end(2, u_done[0])
    emit_combine_end(3, u_done[1])


def _mk_io(nc):
    io = {}
    io["xqb2"] = nc.dram_tensor("xqb2", [128, NQ], BF16,
                                kind="ExternalInput").ap()
    io["ptd"] = nc.dram_tensor("ptd", [128, NRT * 64], BF16,
                               kind="ExternalInput").ap()
    io["kp2d"] = nc.dram_tensor("kp2d", [128, HALF], BF16,
                                kind="ExternalInput").ap()
    io["vtd"] = nc.dram_tensor("vtd", [128, NRT * 66], BF16,
                               kind="ExternalInput").ap()
    io["wpcbd"] = nc.dram_tensor("wpcbd", [128, NRT], F32,
                                 kind="ExternalInput").ap()
    io["gc"] = nc.dram_tensor("gc", [64, 1], F32, kind="ExternalInput").ap()
    io["eye2"] = nc.dram_tensor("eye2", [64, 64], F32,
                                kind="ExternalInput").ap()
    io["out"] = nc.dram_tensor("out", [64, NQ], F32,
                               kind="ExternalOutput").ap()
    return io


_CACHE = {}


def build_program():
    if "nc" not in _CACHE:
        nc = bacc.Bacc("TRN2", target_bir_lowering=False, debug=False,
                       num_devices=NCORES)
        io = _mk_io(nc)
        with tile.TileContext(nc) as tc, ExitStack() as ctx:
            build_danet(ctx, tc, io)
        nc.compile()
        _CACHE["nc"] = nc
    return _CACHE["nc"]


def make_in_maps(x, Wq, bq, Wk, bk, Wv, bv, gamma_c, gamma_p):
    f = np.float32
    bf = ml_dtypes.bfloat16
    proj = np.asarray(x, f).reshape(B, C, N)
    Wq, bq, Wk, bk = (np.asarray(a, f) for a in (Wq, bq, Wk, bk))
    Wv, bv = np.asarray(Wv, f), np.asarray(bv, f)
    gamma_c = float(np.asarray(gamma_c).reshape(-1)[0])
    gamma_p = float(np.asarray(gamma_p).reshape(-1)[0])

    M = Wq.T @ Wk                        # rank-32 score matrix
    w = Wk.T @ bq                        # per-key bias inside softmax
    gc = np.full((64, 1), gamma_c, f)
    eye2 = (2.0 * np.eye(64)).astype(f)
    # vt tiles in pair order [0,32,1,33,...,30,62,31] so a DMA prefix
    # covers the first flash pairs
    perm = []
    for i in range(32):
        perm.append(i)
        if 32 + i <= NRT - 1:
            perm.append(32 + i)

    in_maps = []
    for core in range(NCORES):
        b, qb = divmod(core, 4)
        pp = np.zeros((64, NRT * MT), f)
        pp[:, 0:N] = proj[b]
        kp = np.zeros((64, NPAD), f)     # zero on pad keys
        kp[:, 0:N] = M @ proj[b]
        kp2d = np.concatenate([kp[:, 0:HALF], kp[:, HALF:NPAD]], axis=0)
        vtt = np.zeros((NRT * MT, 66), f)
        vtt[0:N, 0:64] = (gamma_p * (Wv @ proj[b] + bv[:, None])).T
        vtt[0:N, 64] = 1.0
        wp = np.zeros(NRT * MT, f)
        wp[0:N] = w @ proj[b]
        vtt[:, 65] = wp
        vtt = vtt.reshape(NRT, MT, 66)[perm]          # pair order
        vtd = np.ascontiguousarray(
            vtt.transpose(1, 0, 2).reshape(MT, NRT * 66))
        wpcb = (wp * SCH_C + SCH_B).reshape(NRT, MT)[perm]
        wpcbd = np.ascontiguousarray(wpcb.T)          # [128, NRT]
        ptd = np.ascontiguousarray(
            pp.reshape(64, NRT, MT).transpose(2, 1, 0).reshape(MT, NRT * 64))
        xqf = np.ascontiguousarray(proj[b][:, qb * NQ:(qb + 1) * NQ])
        xqb2 = np.broadcast_to(xqf.astype(bf), (2, 64, NQ)).reshape(128, NQ)
        in_maps.append({"xqb2": np.ascontiguousarray(xqb2),
                        "ptd": ptd.astype(bf), "kp2d": kp2d.astype(bf),
                        "vtd": vtd.astype(bf), "wpcbd": wpcbd.astype(f),
                        "gc": gc, "eye2": eye2})
    return in_maps


def run_on_cores(in_maps, **kw):
    nc = build_program()
    return run_bass_kernel_spmd(nc, in_maps, core_ids=list(range(NCORES)),
                                **kw)


def kernel(**inputs):
    x = np.asarray(inputs["x"])
    in_maps = make_in_maps(
        inputs["x"], inputs["Wq"], inputs["bq"], inputs["Wk"], inputs["bk"],
        inputs["Wv"], inputs["bv"], inputs["gamma_c"], inputs["gamma_p"])
    res = run_on_cores(in_maps)
    out = np.zeros((B, C, N), np.float32)
    for core in range(NCORES):
        b, qb = divmod(core, 4)
        out[b][:, qb * NQ:(qb + 1) * NQ] = res.results[core]["out"]
    return out.reshape(x.shape).astype(x.dtype, copy=False)
